# revision 10
# baseline (speedup 1.0000x reference)
"""GAT (2-layer) + MLP head on 8 TRN2 NeuronCores.

Strategy
--------
The random edge list (320k edges over 1600 nodes) is converted on the host
into a dense edge-count matrix C [dst, src] (a lossless re-layout of
edge_index: C[d,s] = number of (s->d) edges).  The GAT edge softmax then
becomes dense ops + matmuls.  Key algebraic trick: exp is monotone, so

    exp(leaky_relu(asrc+adst)) = max(exp(asrc+adst), exp(0.2(asrc+adst)))
                               = exp(adst) * max(u[s], u2[s]*w[d])

with u = exp(asrc), u2 = exp(0.2 asrc), w = exp(-0.8 adst).  The per-dst
factor exp(adst) cancels in the softmax, so the attention weights are

    P[s,d] = C[s,d] * max(u[s], u2[s]*w[d])

i.e. one fused tensor_scalar (mult+max, per-partition scalars) per
(head, k-tile) plus one big tensor_tensor multiply by C per head -- no
pair-space exp/prelu at all.  Exps run only on per-node vectors.

Sharding: each core owns 25 destination nodes of each of the 8 graphs
(dst-interleaved).  That makes the final FC layer column-shardable with a
tiny [8,200] AllReduce, while layer1->layer2 needs an AllGather of the
hidden state.  W2 (layer-2 GAT weight) is folded into the FC1 weight on
the host (out2 = tnorm @ W2 commutes into z = tnorm_flat @ (W2-folded
fc1)).

Matmuls run in bf16 with fp32 PSUM accumulation; the layer-2 aggregation
runs fp8 DoubleRow (2 k-tiles per instruction) since both operands (the
gathered hidden state and the attention weights pt2) tolerate fp8.

Input DMA priority: only xTd/Wa/xT/W1/Ct1 (~1.6 MB) gate the start of
layer 1; Ct2 and especially WfcT (5.1 MB, needed only ~100us later) are
issued last so they don't crowd HBM bandwidth at kernel start.
"""

import os
import sys
import numpy as np

sys.path.insert(0, "/opt/trn_rl_repo")

import ml_dtypes  # noqa: E402

import concourse.bass as bass  # noqa: E402
from concourse import bacc  # noqa: E402
from concourse import mybir  # noqa: E402
from concourse.tile import TileContext  # noqa: E402
from concourse.bass_utils import run_bass_kernel_spmd  # noqa: E402

# ---------------------------------------------------------------- constants
N = 1600
ROI = 200
HID = 64
HIN = 8
D1 = HID * HIN  # 512
B = 8
NCORES = 8
NEG = 0.2
NODES_PER_CORE = N // NCORES       # 200
PER_GRAPH = NODES_PER_CORE // B    # 25

F32 = mybir.dt.float32
BF16 = mybir.dt.bfloat16
F8 = mybir.dt.float8e4

DEBUG_STAGE = os.environ.get("KERNEL_DEBUG_STAGE") or None

# node k-tiles over the 1600-node dim
KT = [(t * 128, min(128, N - t * 128)) for t in range((N + 127) // 128)]  # 13
NKT = len(KT)

GAW = 514         # gather payload width: 512 feats | ones | asrc2
GAWP = 528        # g1a SBUF row pitch: 16B-aligned for DoubleRow weight APs
PTP = 208         # pt2 SBUF row pitch: 16B-aligned for DoubleRow moving APs

_BF = ml_dtypes.bfloat16


def _bf(x):
    return np.ascontiguousarray(x.astype(_BF))


def _f32(x):
    return np.ascontiguousarray(x.astype(np.float32))


def _ap_cols(ap, start, stride, count):
    """Sub-AP selecting `count` columns with `stride` from a 2D [P, F] AP."""
    return bass.AP(
        tensor=ap.tensor,
        offset=ap.offset + start * ap.ap[-1][0],
        ap=[ap.ap[0], [ap.ap[-1][0] * stride, count]],
    )


def _dram_bcast(handle, n_part, offset, stride, count):
    """DRAM read AP replicating a strided 1-D slice across n_part partitions."""
    return bass.AP(
        tensor=handle, offset=offset, ap=[[0, n_part], [stride, count]]
    )


# ---------------------------------------------------------------- program
def build_program():
    nc = bacc.Bacc("TRN2", num_devices=NCORES)

    # ---- I/O ----
    d_xT = nc.dram_tensor("xT", [100, 2, N], BF16, kind="ExternalInput")
    d_xTd = nc.dram_tensor("xTd", [100, 2, NODES_PER_CORE], BF16, kind="ExternalInput")
    d_W1 = nc.dram_tensor("W1", [100, 2, D1], BF16, kind="ExternalInput")
    d_Wa = nc.dram_tensor("Wa", [100, 2, 16], BF16, kind="ExternalInput")
    d_Ct1 = nc.dram_tensor("Ct1", [128, NKT, NODES_PER_CORE], BF16, kind="ExternalInput")
    d_Ct2 = nc.dram_tensor("Ct2", [128, NKT, NODES_PER_CORE], BF16, kind="ExternalInput")
    d_b1 = nc.dram_tensor("b1", [D1], F32, kind="ExternalInput")
    d_Wb = nc.dram_tensor("Wb", [2, D1], F32, kind="ExternalInput")
    d_WfcT = nc.dram_tensor("WfcT", [128, 100, ROI], BF16, kind="ExternalInput")
    d_sel = nc.dram_tensor("sel", [128, B], F32, kind="ExternalInput")
    d_bnsc8 = nc.dram_tensor("bnsc8", [B, ROI], F32, kind="ExternalInput")
    d_bnsh8 = nc.dram_tensor("bnsh8", [B, ROI], F32, kind="ExternalInput")
    d_fc2w8 = nc.dram_tensor("fc2w8", [B, 2, ROI], F32, kind="ExternalInput")
    d_fc2b8 = nc.dram_tensor("fc2b8", [B, 2], F32, kind="ExternalInput")
    d_out = nc.dram_tensor("logits", [B, 2], F32, kind="ExternalOutput")
    d_dbg = nc.dram_tensor("dbg", [128, 800], F32, kind="ExternalOutput")

    # ---- collective buffers ----
    d_ag_in = nc.dram_tensor("ag_in", [NODES_PER_CORE, GAW], F8, kind="Internal")
    d_ag_out = nc.dram_tensor(
        "ag_out", [N, GAW], F8, kind="Internal", addr_space="Shared"
    )
    d_warm_in = nc.dram_tensor("warm_in", [1, 8], F8, kind="Internal")
    d_warm_out = nc.dram_tensor(
        "warm_out", [NCORES, 8], F8, kind="Internal", addr_space="Shared"
    )
    d_wdst = nc.dram_tensor("wdst", [HIN, NODES_PER_CORE], BF16, kind="Internal")
    d_w2dst = nc.dram_tensor("w2dst", [NODES_PER_CORE], BF16, kind="Internal")
    d_ar_in = nc.dram_tensor("ar_in", [B, ROI], F32, kind="Internal")
    d_ar_out = nc.dram_tensor(
        "ar_out", [B, ROI], F32, kind="Internal", addr_space="Shared"
    )

    groups = [list(range(NCORES))]

    with TileContext(nc) as tc:
        _build_body(nc, tc, locals())

    nc.finalize()

    in_names = [
        "xT", "xTd", "W1", "Wa", "Ct1", "Ct2", "b1", "Wb",
        "WfcT", "sel", "bnsc8", "bnsh8", "fc2w8", "fc2b8",
    ]
    return nc, in_names


def _build_body(nc, tc, d):
    from contextlib import ExitStack

    d_xT = d["d_xT"]; d_xTd = d["d_xTd"]; d_W1 = d["d_W1"]; d_Wa = d["d_Wa"]
    d_Ct1 = d["d_Ct1"]; d_Ct2 = d["d_Ct2"]; d_b1 = d["d_b1"]
    d_Wb = d["d_Wb"]; d_WfcT = d["d_WfcT"]; d_sel = d["d_sel"]
    d_bnsc8 = d["d_bnsc8"]; d_bnsh8 = d["d_bnsh8"]
    d_fc2w8 = d["d_fc2w8"]; d_fc2b8 = d["d_fc2b8"]; d_out = d["d_out"]
    d_ag_in = d["d_ag_in"]; d_ag_out = d["d_ag_out"]
    d_warm_in = d["d_warm_in"]; d_warm_out = d["d_warm_out"]
    d_wdst = d["d_wdst"]; d_w2dst = d["d_w2dst"]
    d_ar_in = d["d_ar_in"]; d_ar_out = d["d_ar_out"]
    d_dbg = d["d_dbg"]
    groups = d["groups"]

    ACT = mybir.ActivationFunctionType
    ALU = mybir.AluOpType
    DR = mybir.MatmulPerfMode.DoubleRow

    def _dbg_out(work, src_ap):
        dbg = work.tile([B, 2], F32, tag="dbgo", name="dbgo")
        nc.vector.tensor_copy(dbg, src_ap)
        nc.sync.dma_start(out=d_out[:], in_=dbg)

    with ExitStack() as ctx:
        singles = ctx.enter_context(tc.tile_pool(name="singles", bufs=1))
        work = ctx.enter_context(tc.tile_pool(name="work", bufs=3))

        # ------------------------------------------------ static loads
        ones_row = singles.tile([1, 128], BF16)
        nc.vector.memset(ones_row, 1.0)
        # No warm-up collective: the CC subsystem takes ~78us from kernel
        # start to become ready regardless of trigger time, and each mesh
        # blocks the next trigger for ~20us after it ends.  Layer 1 finishes
        # right around the ready point, so the real AllGather IS the warm-up.

        # phase-A-critical loads split across the two HWDGE rings; each ring
        # drains in issue order, so the critical tensors arrive first.
        # Ct2 / WfcT (needed ~100us later) are issued LAST so they don't
        # crowd HBM bandwidth while phase A waits on xT/W1/Ct1.
        xTd = singles.tile([100, 2, NODES_PER_CORE], BF16)
        nc.sync.dma_start(out=xTd[:], in_=d_xTd[:])
        Wa = singles.tile([100, 2, 16], BF16)
        nc.sync.dma_start(out=Wa[:], in_=d_Wa[:])
        xT = singles.tile([100, 2, N], BF16)           # x^T k-tiles (K=200=2x100)
        for ci, (c0, c1) in enumerate(((0, 512), (512, 1024), (1024, 1536), (1536, N))):
            eng = nc.sync if ci % 2 == 0 else nc.scalar
            eng.dma_start(out=xT[:, :, c0:c1], in_=d_xT[:, :, c0:c1])
        W1 = singles.tile([100, 2, D1], BF16)
        nc.sync.dma_start(out=W1[:], in_=d_W1[:])

        Ct1 = singles.tile([128, NKT, NODES_PER_CORE], BF16)
        Ct2 = singles.tile([128, NKT, NODES_PER_CORE], BF16)
        nc.sync.dma_start(out=Ct1[:, 0:7, :], in_=d_Ct1[:, 0:7, :])
        nc.scalar.dma_start(out=Ct1[:, 7:NKT, :], in_=d_Ct1[:, 7:NKT, :])

        b1b = singles.tile([128, D1], F32)  # b1 broadcast across partitions
        nc.sync.dma_start(out=b1b, in_=_dram_bcast(d_b1, 128, 0, 1, D1))
        Wbb = singles.tile([128, 2, D1], F32)  # wsrc2 / wdst2 broadcast
        nc.sync.dma_start(
            out=Wbb,
            in_=bass.AP(tensor=d_Wb, offset=0, ap=[[0, 128], [D1, 2], [1, D1]]),
        )
        WfcT = singles.tile([128, 100, ROI], BF16)
        sel = singles.tile([128, B], F32)
        bnsc8 = singles.tile([B, ROI], F32)
        bnsh8 = singles.tile([B, ROI], F32)
        fc2w8 = singles.tile([B, 2, ROI], F32)
        fc2b8 = singles.tile([B, 2], F32)

        # small tail constants early (cheap), then the big late tensors last.
        # The late bulk (Ct2, WfcT -- 5.8 MB needed only ~100us in) rides the
        # scalar ring ONLY: the sync ring must stay clear for mid-kernel DMAs
        # (wdst round trip, gather payload, g1a loads) which would otherwise
        # queue behind these in ring-FIFO order.
        nc.scalar.dma_start(out=sel[:], in_=d_sel[:])
        nc.scalar.dma_start(out=bnsc8[:], in_=d_bnsc8[:])
        nc.scalar.dma_start(out=bnsh8[:], in_=d_bnsh8[:])
        nc.scalar.dma_start(out=fc2w8[:], in_=d_fc2w8[:])
        nc.scalar.dma_start(out=fc2b8[:], in_=d_fc2b8[:])
        nc.scalar.dma_start(out=Ct2[:], in_=d_Ct2[:])
        for wc in range(4):
            nc_slice = slice(wc * 25, (wc + 1) * 25)
            nc.scalar.dma_start(out=WfcT[:, nc_slice, :], in_=d_WfcT[:, nc_slice, :])

        # ------------------------------------------------ phase A: asdT1 + h1
        h1s = singles.tile([128, NKT, HIN, HID + 1], BF16)
        nc.vector.memset(h1s[:, :, :, HID : HID + 1], 1.0)
        # PE warm-up burst: junk matmuls during the input-DMA wait push
        # the HAM clock gate up before the real matmuls start.
        with tc.tile_pool(name="pW", bufs=1, space="PSUM") as pW:
            psW = pW.tile([128, 128], F32, tag="psW")
            for i in range(24):
                nc.tensor.matmul(
                    psW, ones_row, ones_row, start=(i == 0), stop=(i == 23)
                )
        asd1 = singles.tile([128, NKT, HIN], F32)   # asrc1 for all nodes
        nc.vector.memset(asd1[64:128, NKT - 1, :], 0.0)  # pad tile garbage guard
        ue = singles.tile([128, NKT, HIN], F32)     # exp(asrc1)
        ue2 = singles.tile([128, NKT, HIN], F32)    # exp(0.2*asrc1)
        wb = singles.tile([128, HIN, NODES_PER_CORE], BF16)  # exp(-0.8*adst1) bcast
        with tc.tile_pool(name="pA", bufs=2, space="PSUM") as pA:
            # asdT1 = Wa^T @ x[D_j]^T  (row h = asrc1_h, row 8+h = adst1_h)
            ps_asdT = pA.tile([16, NODES_PER_CORE], F32)
            for kc in range(2):
                nc.tensor.matmul(
                    ps_asdT, Wa[:, kc, :], xTd[:, kc, :],
                    start=(kc == 0), stop=(kc == 1),
                )
            # w_h[d] = exp(-0.8 * adst1_h[d]), broadcast across 128 partitions
            wloc = work.tile([16, NODES_PER_CORE], BF16, tag="wloc")
            nc.scalar.activation(wloc, ps_asdT, ACT.Exp, scale=-0.8)
            nc.sync.dma_start(out=d_wdst[:], in_=wloc[8:16, :])
            nc.sync.dma_start(
                out=wb[:],
                in_=bass.AP(
                    tensor=d_wdst,
                    offset=0,
                    ap=[[0, 128], [NODES_PER_CORE, HIN], [1, NODES_PER_CORE]],
                ),
            )

            # attention-score matmuls first so ue/ue2 (which gate the whole
            # layer-1 DVE producer chain) are ready before the h1 matmuls run
            for k, (k0, pk) in enumerate(KT):
                ps_a = pA.tile([128, HIN], F32, tag="ps_a")
                for kc in range(2):
                    nc.tensor.matmul(
                        ps_a[0:pk, :],
                        xT[:, kc, k0 : k0 + pk],
                        Wa[:, kc, 0:HIN],
                        start=(kc == 0),
                        stop=(kc == 1),
                    )
                nc.scalar.copy(asd1[0:pk, k, :], ps_a[0:pk, :])
                if k in (3, 7, 11, 12):
                    ka = {3: 0, 7: 4, 11: 8, 12: 12}[k]
                    nc.scalar.activation(
                        ue[:, ka : k + 1, :], asd1[:, ka : k + 1, :], ACT.Exp
                    )
                    nc.scalar.activation(
                        ue2[:, ka : k + 1, :], asd1[:, ka : k + 1, :],
                        ACT.Exp, scale=NEG,
                    )

            # h1 = x @ W1;  h1s[:, k, h, 64] stays 1.0 (denominator col)
            for k, (k0, pk) in enumerate(KT):
                ps_h = pA.tile([128, D1], F32, tag="ps_h")
                for kc in range(2):
                    nc.tensor.matmul(
                        ps_h[0:pk, :],
                        xT[:, kc, k0 : k0 + pk],
                        W1[:, kc, :],
                        start=(kc == 0),
                        stop=(kc == 1),
                    )
                pa = ps_h[0:pk, :]
                nc.scalar.copy(
                    h1s[0:pk, k, :, 0:HID],
                    bass.AP(
                        tensor=pa.tensor,
                        offset=pa.offset,
                        ap=[pa.ap[0], [HID, HIN], [1, HID]],
                    ),
                )

        if DEBUG_STAGE == "h1":
            dbh = work.tile([128, 536], F32, tag="dbh", name="dbh")
            nc.vector.tensor_copy(dbh[:, 0:520], h1s[:, 0, :, :])
            nc.vector.tensor_copy(dbh[:, 520:528], ue[:, 0, :])
            nc.vector.tensor_copy(dbh[:, 528:536], ue2[:, 0, :])
            nc.sync.dma_start(out=d_dbg[:, 0:536], in_=dbh)
            _dbg_out(work, ue[0:B, 0, 0:2])
            return

        # ------------------------------------------------ layer-1 attention
        g1f = singles.tile([100, 2, D1], F32)  # post-attention hidden state
        asd2acc = singles.tile([100, 2, 2, 3], F32)  # [m, vi, chunk] partial dots
        g1o1 = singles.tile([100, 2, 256], F8)
        g1o2 = singles.tile([100, 2, 258], F8)
        nc.vector.memset(g1o2[:, :, 256:257], 1.0)   # ones col for L2 denominator
        with tc.tile_pool(name="pL1", bufs=2, space="PSUM") as pL1:
            for h in range(HIN):
                # pair-space attention weights:
                #   pt[s,d] = Ct1[s,d] * max(u[s], u2[s]*w[d])
                m2 = work.tile([128, NKT, NODES_PER_CORE], BF16, tag="m2", bufs=2)
                for k, (k0, pk) in enumerate(KT):
                    nc.vector.tensor_scalar(
                        m2[0:pk, k, :],
                        wb[0:pk, h, :],
                        ue2[0:pk, k, h : h + 1],
                        ue[0:pk, k, h : h + 1],
                        ALU.mult,
                        ALU.max,
                    )
                pt = work.tile([128, NKT, NODES_PER_CORE], BF16, tag="pt", bufs=2)
                nc.vector.tensor_mul(pt[:], m2[:], Ct1[:])

                psA = pL1.tile([100, HID + 1], F32, tag="psA")
                psB = pL1.tile([100, HID + 1], F32, tag="psB")
                for k, (k0, pk) in enumerate(KT):
                    nc.tensor.matmul(
                        psA,
                        pt[0:pk, k, 0:100],
                        h1s[0:pk, k, h, :],
                        start=(k == 0),
                        stop=(k == NKT - 1),
                    )
                    nc.tensor.matmul(
                        psB,
                        pt[0:pk, k, 100:200],
                        h1s[0:pk, k, h, :],
                        start=(k == 0),
                        stop=(k == NKT - 1),
                    )
                if DEBUG_STAGE == "l1m":
                    if h == 0:
                        _dbg_out(work, psA[0:B, 0:2])
                    continue
                for m, ps in ((0, psA), (1, psB)):
                    rec = work.tile([100, 1], F32, tag="rec")
                    nc.vector.reciprocal(rec, ps[:, HID : HID + 1])
                    # divide by softmax denominator on the scalar engine
                    nc.scalar.activation(
                        g1f[:, m, h * HID : (h + 1) * HID],
                        ps[:, 0:HID],
                        ACT.Copy,
                        scale=rec[:, 0:1],
                    )
                if h in (3, 6, HIN - 1):
                    # completed feature chunk: +b1, ELU, asd2 partial dots,
                    # then DMA into the gather payload.  Chunked at heads
                    # 3/6/7 so the tail chunk (64 cols) before the AllGather
                    # trigger is short.
                    ci = (3, 6, 7).index(h)
                    c0, c1 = ((0, 256), (256, 448), (448, 512))[ci]
                    cw = c1 - c0
                    for m in range(2):
                        vv = g1f[:, m, c0:c1]
                        nc.vector.tensor_add(vv, vv, b1b[0:100, c0:c1])
                        eneg = work.tile([100, 256], F32, tag="eneg")
                        en = eneg[:, 0:cw]
                        if ci < 2:
                            rpos = work.tile([100, 256], F32, tag="rpos")
                            rp = rpos[:, 0:cw]
                            nc.scalar.activation(en, vv, ACT.Relu, scale=-1.0)
                            nc.scalar.activation(en, en, ACT.Exp, scale=-1.0)
                            nc.scalar.activation(rp, vv, ACT.Relu)
                            nc.vector.scalar_tensor_tensor(
                                vv, en, -1.0, rp, ALU.add, ALU.add
                            )
                        else:
                            nc.vector.tensor_scalar_min(en, vv, 0.0)
                            nc.scalar.activation(en, en, ACT.Exp)
                            nc.vector.tensor_scalar(vv, vv, 0.0, -1.0, ALU.max, ALU.add)
                            nc.vector.tensor_add(vv, vv, en)
                        # partial asd2 dots over this feature chunk
                        sc2 = work.tile([100, 256], F32, tag="sc2")
                        for vi in range(2):
                            nc.vector.scalar_tensor_tensor(
                                sc2[:, 0:cw],
                                vv,
                                1.0,
                                Wbb[0:100, vi, c0:c1],
                                ALU.mult,
                                ALU.mult,
                                accum_out=asd2acc[:, m, vi, ci : ci + 1],
                            )
                        if ci == 0:
                            nc.scalar.copy(g1o1[:, m, :], vv)
                            nc.sync.dma_start(
                                out=d_ag_in[m * 100 : (m + 1) * 100, 0:256],
                                in_=g1o1[:, m, :],
                            )
                        else:
                            nc.scalar.copy(g1o2[:, m, c0 - 256 : c1 - 256], vv)
                            if ci == 1:
                                nc.sync.dma_start(
                                    out=d_ag_in[m * 100 : (m + 1) * 100, 256:448],
                                    in_=g1o2[:, m, 0:192],
                                )

        if DEBUG_STAGE == "l1m":
            return

        # ------------------------------------------------ g1 post: asd2 + AG2
        w2loc = work.tile([100, 2, 1], BF16, tag="w2loc")
        for m in range(2):
            asd2 = work.tile([100, 2], F32, tag="asd2")
            nc.vector.tensor_add(
                asd2, asd2acc[:, m, :, 0], asd2acc[:, m, :, 1]
            )
            nc.vector.tensor_add(asd2, asd2, asd2acc[:, m, :, 2])
            # asrc2 -> gather payload col 257
            nc.vector.tensor_copy(g1o2[:, m, 257:258], asd2[:, 0:1])
            # w2[d] = exp(-0.8 * adst2[d]) (kept local)
            nc.scalar.activation(w2loc[:, m, :], asd2[:, 1:2], ACT.Exp, scale=-0.8)
            nc.sync.dma_start(
                out=d_ag_in[m * 100 : (m + 1) * 100, 448:GAW],
                in_=g1o2[:, m, 192:258],
            )
            nc.sync.dma_start(
                out=d_w2dst[m * 100 : (m + 1) * 100], in_=w2loc[:, m, :]
            )

        if DEBUG_STAGE == "l1":
            nc.sync.dma_start(out=d_dbg[0:100, 0:512], in_=g1f[:, 0, :])
            _dbg_out(work, g1f[0:B, 0, 0:2])
            return

        nc.gpsimd.collective_compute(
            "AllGather",
            ALU.bypass,
            replica_groups=groups,
            ins=[d_ag_in[:]],
            outs=[d_ag_out[:]],
        )

        # gathered hidden state: g1a [128, k, 514] = [feat 512 | ones | asrc2].
        # Read + process per k-tile so layer-2 pipeline starts on tile 0
        # while later tiles are still in flight.
        # A dummy read gated on the AllGather feeds a short junk-matmul burst:
        # the PE sits idle ~25us during the collective, and layer 2 would
        # start cold -- the burst re-warms it just before the real matmuls.
        agdum = singles.tile([1, 64], F8)
        nc.sync.dma_start(out=agdum, in_=d_ag_out[0:1, 0:64])
        with tc.tile_pool(name="pW2", bufs=1, space="PSUM") as pW2:
            psW2 = pW2.tile([128, 64], F32, tag="psW2")
            for i in range(14):
                nc.tensor.matmul(
                    psW2, ones_row, agdum, start=(i == 0), stop=(i == 13)
                )
        g1a = singles.tile([128, NKT, GAWP], F8)

        # own shard's w2 broadcast across partitions
        wb2 = singles.tile([128, NODES_PER_CORE], BF16)
        nc.sync.dma_start(
            out=wb2,
            in_=bass.AP(
                tensor=d_w2dst, offset=0, ap=[[0, 128], [1, NODES_PER_CORE]]
            ),
        )
        asrc2f = singles.tile([128, NKT], F32)
        Ue = singles.tile([128, NKT], F32)
        Ue2 = singles.tile([128, NKT], F32)
        nc.vector.memset(asrc2f[64:128, NKT - 1 : NKT], 0.0)  # pad guard
        for k, (k0, pk) in enumerate(KT):
            nc.sync.dma_start(
                out=g1a[0:pk, k, 0:GAW],
                in_=bass.AP(
                    tensor=d_ag_out,
                    offset=k * 128 * GAW,
                    ap=[[GAW, pk], [1, GAW]],
                ),
            )
            nc.scalar.copy(asrc2f[0:pk, k : k + 1], g1a[0:pk, k, GAW - 1 : GAW])
            nc.scalar.activation(
                Ue[0:pk, k : k + 1], asrc2f[0:pk, k : k + 1], ACT.Exp
            )
            nc.scalar.activation(
                Ue2[0:pk, k : k + 1], asrc2f[0:pk, k : k + 1], ACT.Exp, scale=NEG
            )

        if DEBUG_STAGE == "ag":
            dba = work.tile([128, 514], F32, tag="dba", name="dba")
            nc.vector.tensor_copy(dba, g1a[:, 0, 0:GAW])
            nc.sync.dma_start(out=d_dbg[:, 0:514], in_=dba)
            _dbg_out(work, g1a[0:B, 0, 0:2])
            return

        # ------------------------------------------------ layer-2 attention
        # attention weights pt2 in fp8 so the aggregation matmuls run
        # DoubleRow (2 k-tiles per instruction, half the cycles/row).
        tnorm = singles.tile([128, 4, NODES_PER_CORE], BF16)
        with tc.tile_pool(name="pL2", bufs=1, space="PSUM") as pL2:
            m22 = work.tile([128, NKT, NODES_PER_CORE], BF16, tag="m22", bufs=1)
            pt2 = work.tile([128, NKT, PTP], F8, tag="pt2", bufs=1)
            psT = [
                pL2.tile([128, NODES_PER_CORE], F32, tag=f"psT{c}", name=f"psT{c}")
                for c in range(4)
            ]
            psD = pL2.tile([1, NODES_PER_CORE], F32, tag="psD")
            for k, (k0, pk) in enumerate(KT):
                nc.vector.tensor_scalar(
                    m22[0:pk, k, :],
                    wb2[0:pk, :],
                    Ue2[0:pk, k : k + 1],
                    Ue[0:pk, k : k + 1],
                    ALU.mult,
                    ALU.max,
                )
                nc.vector.tensor_mul(
                    pt2[0:pk, k, 0:NODES_PER_CORE], m22[0:pk, k, :], Ct2[0:pk, k, :]
                )
                if k % 2 == 1:
                    # DoubleRow pair (k-1, k): both operands fp8
                    for c in range(4):
                        nc.tensor.matmul(
                            psT[c],
                            g1a[:, k - 1 : k + 1, c * 128 : (c + 1) * 128],
                            pt2[:, k - 1 : k + 1, 0:NODES_PER_CORE],
                            start=(k == 1),
                            stop=False,
                            perf_mode=DR,
                        )
                    nc.tensor.matmul(
                        psD,
                        g1a[:, k - 1 : k + 1, D1 : D1 + 1],
                        pt2[:, k - 1 : k + 1, 0:NODES_PER_CORE],
                        start=(k == 1),
                        stop=False,
                        perf_mode=DR,
                    )
                elif k == NKT - 1:
                    for c in range(4):
                        nc.tensor.matmul(
                            psT[c],
                            g1a[0:pk, k, c * 128 : (c + 1) * 128],
                            pt2[0:pk, k, 0:NODES_PER_CORE],
                            start=False,
                            stop=True,
                        )
                    nc.tensor.matmul(
                        psD,
                        g1a[0:pk, k, D1 : D1 + 1],
                        pt2[0:pk, k, 0:NODES_PER_CORE],
                        start=False,
                        stop=True,
                    )

            # denominator reciprocal, broadcast across partitions via PE
            d2 = work.tile([1, NODES_PER_CORE], F32, tag="d2")
            nc.vector.reciprocal(d2, psD[0:1, :])
            d2b = work.tile([1, NODES_PER_CORE], BF16, tag="d2b")
            nc.vector.tensor_copy(d2b, d2)
            ps_rb = pL2.tile([128, NODES_PER_CORE], F32, tag="ps_rb")
            nc.tensor.matmul(ps_rb, ones_row, d2b)
            rb = work.tile([128, NODES_PER_CORE], F32, tag="rb")
            nc.scalar.copy(rb, ps_rb)

            # normalized layer-2 message sums (pre-W2; W2 folded into WfcT).
            # chunk 0 first so FC1's first column group can start while the
            # other chunks normalize.
            for c in range(4):
                nc.vector.tensor_mul(tnorm[:, c, :], psT[c], rb)

                if DEBUG_STAGE == "l2" and c == 0:
                    break

        if DEBUG_STAGE == "l2":
            dbo = work.tile([128, 600], F32, tag="dbo", name="dbo")
            nc.vector.tensor_copy(dbo[:, 0:200], tnorm[:, 0, :])
            nc.sync.dma_start(out=d_dbg[:, 0:600], in_=dbo)
            _dbg_out(work, tnorm[0:B, 0, 0:2])
            return

        # ------------------------------------------------ FC1 partial + AllReduce
        # 100 k-slice matmuls with M=8 packed 4-wide into PE column groups;
        # the 4 groups are summed on-chip (selector matmul) before a small
        # [8,200] AllReduce.
        with tc.tile_pool(name="pFC", bufs=1, space="PSUM") as pFC:
            psZ4 = pFC.tile([128, ROI], F32, tag="psZ4")
            order = [c for fcch in range(4) for c in range(fcch, 100, 4)]
            counts = [0, 0, 0, 0]
            for i, c in enumerate(order):
                j = i % 4
                l = c // 4
                fcch = c % 4
                stat = _ap_cols(tnorm[:, fcch, :], l, PER_GRAPH, B)
                nc.tensor.matmul(
                    psZ4[32 * j : 32 * j + B, :],
                    stat,
                    WfcT[:, c, :],
                    start=(counts[j] == 0),
                    stop=(counts[j] == 24),
                    tile_position=(0, 32 * j),
                    skip_group_check=True,
                )
                counts[j] += 1
            # sum the 4 column groups on-chip: psZs = sel^T @ zsb
            zsb = work.tile([128, ROI], F32, tag="zsb")
            nc.vector.tensor_copy(zsb, psZ4)
            psZs = pFC.tile([B, ROI], F32, tag="psZs")
            nc.tensor.matmul(psZs, sel, zsb)
            zs8 = work.tile([B, ROI], F32, tag="zs8")
            nc.vector.tensor_copy(zs8, psZs)
            nc.sync.dma_start(out=d_ar_in[:], in_=zs8)

            if DEBUG_STAGE == "fc":
                nc.sync.dma_start(out=d_dbg[0:B, 0:200], in_=zs8[0:B, :])
                _dbg_out(work, zs8[0:B, 0:2])
                return

            nc.gpsimd.collective_compute(
                "AllReduce",
                ALU.add,
                replica_groups=groups,
                ins=[d_ar_in[:]],
                outs=[d_ar_out[:]],
            )

            # ------------------------------------------------ BN + ELU + FC2
            # contiguous [8,200] read; BN via host-pre-broadcast scale/shift;
            # FC2 as two DVE accumulation ops (no transpose round-trip).
            zt = work.tile([B, ROI], F32, tag="zt")
            nc.sync.dma_start(out=zt, in_=d_ar_out[:])
            nc.vector.tensor_mul(zt, zt, bnsc8)
            nc.vector.tensor_add(zt, zt, bnsh8)
            en = work.tile([B, ROI], F32, tag="en")
            nc.vector.tensor_scalar_min(en, zt, 0.0)
            nc.scalar.activation(en, en, ACT.Exp)
            nc.vector.tensor_scalar(zt, zt, 0.0, -1.0, ALU.max, ALU.add)
            nc.vector.tensor_add(zt, zt, en)
            lg = work.tile([B, 2], F32, tag="lg")
            junk = work.tile([B, ROI], F32, tag="junk")
            for c in range(2):
                nc.vector.scalar_tensor_tensor(
                    junk, zt, 1.0, fc2w8[:, c, :], ALU.mult, ALU.mult,
                    accum_out=lg[:, c : c + 1],
                )
            lsb = work.tile([B, 2], F32, tag="lsb")
            nc.vector.tensor_add(lsb, lg, fc2b8)
            nc.sync.dma_start(out=d_out[:], in_=lsb)


# ---------------------------------------------------------------- host side
def _prepare_inputs(x, edge_index, W1, a1_src, a1_dst, b1, W2, a2_src, a2_dst,
                    b2, fc1_w, fc1_b, bn_g, bn_b, bn_m, bn_v, fc2_w, fc2_b):
    x = np.asarray(x, np.float32)
    ei = np.asarray(edge_index)
    src, dst = ei[0].astype(np.int64), ei[1].astype(np.int64)
    C = np.bincount(dst * N + src, minlength=N * N).reshape(N, N).astype(np.float32)
    assert C.max() < 256, "edge multiplicity too large for bf16"

    # pi-order: core j owns, for each graph g, within-graph nodes [25j, 25j+25)
    D = [
        np.array(
            [g * ROI + PER_GRAPH * j + k for g in range(B) for k in range(PER_GRAPH)],
            np.int64,
        )
        for j in range(NCORES)
    ]
    perm = np.concatenate(D)

    W1 = np.asarray(W1, np.float32)
    a1_src = np.asarray(a1_src, np.float32)
    a1_dst = np.asarray(a1_dst, np.float32)
    W2 = np.asarray(W2, np.float32)
    a2_src = np.asarray(a2_src, np.float32)
    a2_dst = np.asarray(a2_dst, np.float32)
    fc1_w = np.asarray(fc1_w, np.float32)
    b2 = np.asarray(b2, np.float32)

    # Wa[:, h] = W1[:, 64h:64h+64] @ a1_src[h] ; cols 8..16 same with a1_dst
    W1r = W1.reshape(ROI, HIN, HID)
    Wa = np.concatenate(
        [
            np.einsum("rhf,hf->rh", W1r, a1_src),
            np.einsum("rhf,hf->rh", W1r, a1_dst),
        ],
        axis=1,
    )  # [200, 16]
    Wb = np.stack([W2 @ a2_src[0], W2 @ a2_dst[0]], axis=0)  # [2, 512]

    # z = tnorm_flat @ WF^T + zb;  WF[o,i,f0] = sum_f fc1_wr[o,i,f] W2[f0,f]
    fc1_wr = fc1_w.reshape(ROI, ROI, D1)  # [o, node-in-graph, feat]
    zb = fc1_wr.reshape(ROI, -1) @ np.tile(b2, ROI)  # [200] b2 contribution

    bnscale = np.asarray(bn_g, np.float32) / np.sqrt(np.asarray(bn_v, np.float32) + 1e-5)
    bnshift = (
        np.asarray(bn_b, np.float32)
        + (np.asarray(fc1_b, np.float32) + zb - np.asarray(bn_m, np.float32)) * bnscale
    )

    def _ksw(arr2d, P):
        """[K*P?, F] -> [P, K, F] partition-contiguous swizzle (rows r = k*P + p),
        zero-padding rows beyond the array."""
        R, F = arr2d.shape
        K = (R + P - 1) // P
        out = np.zeros((P, K, F), arr2d.dtype)
        for k in range(K):
            r0, r1 = k * P, min((k + 1) * P, R)
            out[0 : r1 - r0, k, :] = arr2d[r0:r1]
        return out

    xT = _bf(_ksw(x.T, 100))                       # [100, 2, 1600]
    W1_b = _bf(_ksw(W1, 100))                      # [100, 2, 512]
    Wa_b = _bf(_ksw(Wa, 100))                      # [100, 2, 16]

    sel = np.zeros((128, B), np.float32)
    for j in range(4):
        for g in range(B):
            sel[32 * j + g, g] = 1.0
    bnsc8 = _f32(np.tile(bnscale[None, :], (B, 1)))       # [8, 200]
    bnsh8 = _f32(np.tile(bnshift[None, :], (B, 1)))       # [8, 200]
    fc2w8 = _f32(np.tile(np.asarray(fc2_w, np.float32)[None, :, :], (B, 1, 1)))
    fc2b8 = _f32(np.tile(np.asarray(fc2_b, np.float32)[None, :], (B, 1)))

    in_maps = []
    for j in range(NCORES):
        Dj = D[j]
        Ct1 = _bf(_ksw(C[Dj, :].T, 128))           # [128, 13, 200]
        Ct2 = _bf(_ksw(C[np.ix_(Dj, perm)].T, 128))
        xTd = _bf(_ksw(x[Dj, :].T, 100))           # [100, 2, 200]
        # FC1 weight slice with W2 folded in:
        # WF[o, i, f0] = fc1_wr[o, 25j+i, :] @ W2[f0, :]^T
        fc1c = fc1_wr[:, PER_GRAPH * j : PER_GRAPH * (j + 1), :]  # [200, 25, 512]
        WF = (fc1c.reshape(-1, D1) @ W2.T).reshape(ROI, PER_GRAPH, D1)
        WfcT = _bf(
            _ksw(WF.transpose(1, 2, 0).reshape(PER_GRAPH * D1, ROI), 128)
        )                                          # [128, 100, 200]
        in_maps.append(
            {
                "xT": xT,
                "xTd": xTd,
                "W1": W1_b,
                "Wa": Wa_b,
                "Ct1": Ct1,
                "Ct2": Ct2,
                "b1": _f32(np.asarray(b1)),
                "Wb": _f32(Wb),
                "WfcT": WfcT,
                "sel": sel,
                "bnsc8": bnsc8,
                "bnsh8": bnsh8,
                "fc2w8": fc2w8,
                "fc2b8": fc2b8,
            }
        )
    return in_maps


_CACHE = {}


def kernel(**inputs):
    if "nc" not in _CACHE:
        nc, in_names = build_program()
        _CACHE["nc"] = nc
        _CACHE["in_names"] = in_names
    nc = _CACHE["nc"]
    in_maps = _prepare_inputs(**inputs)
    res = run_bass_kernel_spmd(nc, in_maps, core_ids=list(range(NCORES)))
    _CACHE["last_results"] = res
    return np.asarray(res.results[0]["logits"], np.float32)


if __name__ == "__main__":
    import reference

    inp = {k: np.asarray(v) for k, v in reference.setup_inputs().items()}
    out = kernel(**inp)
    exp = np.asarray(reference.reference(**inp))
    err = np.abs(out - exp).max() / (np.abs(exp).max() + 1e-30)
    print("out:", out)
    print("exp:", exp)
    print("rel err:", err)


# revision 17
# speedup vs baseline: 1.2242x; 1.2242x over previous
"""GAT (2-layer) + MLP head on 8 TRN2 NeuronCores.

Strategy
--------
The random edge list (320k edges over 1600 nodes) is converted on the host
into a dense edge-count matrix C [dst, src] (a lossless re-layout of
edge_index: C[d,s] = number of (s->d) edges).  The GAT edge softmax then
becomes dense ops + matmuls.  Key algebraic trick: exp is monotone, so

    exp(leaky_relu(asrc+adst)) = max(exp(asrc+adst), exp(0.2(asrc+adst)))
                               = exp(adst) * max(u[s], u2[s]*w[d])

with u = exp(asrc), u2 = exp(0.2 asrc), w = exp(-0.8 adst).  The per-dst
factor exp(adst) cancels in the softmax, so the attention weights are

    P[s,d] = C[s,d] * max(u[s], u2[s]*w[d])

i.e. one fused tensor_scalar (mult+max, per-partition scalars) per
(head, k-tile) plus one big tensor_tensor multiply by C per head -- no
pair-space exp/prelu at all.  Exps run only on per-node vectors.

Sharding: each core owns 25 destination nodes of each of the 8 graphs
(dst-interleaved).  That makes the final FC layer column-shardable with a
tiny [8,200] AllReduce, while layer1->layer2 needs an AllGather of the
hidden state.  W2 (layer-2 GAT weight) is folded into the FC1 weight on
the host (out2 = tnorm @ W2 commutes into z = tnorm_flat @ (W2-folded
fc1)).

Matmuls run in bf16 with fp32 PSUM accumulation; the layer-2 aggregation
runs fp8 DoubleRow (2 k-tiles per instruction) since both operands (the
gathered hidden state and the attention weights pt2) tolerate fp8.

Input DMA priority: only xTd/Wa/xT/W1/Ct1 (~1.6 MB) gate the start of
layer 1; Ct2 and especially WfcT (5.1 MB, needed only ~100us later) are
issued last so they don't crowd HBM bandwidth at kernel start.
"""

import os
import sys
import numpy as np

sys.path.insert(0, "/opt/trn_rl_repo")

import ml_dtypes  # noqa: E402

import concourse.bass as bass  # noqa: E402
from concourse import bacc  # noqa: E402
from concourse import mybir  # noqa: E402
from concourse.tile import TileContext  # noqa: E402
from concourse.bass_utils import run_bass_kernel_spmd  # noqa: E402

# ---------------------------------------------------------------- constants
N = 1600
ROI = 200
HID = 64
HIN = 8
D1 = HID * HIN  # 512
B = 8
NCORES = 8
NEG = 0.2
NODES_PER_CORE = N // NCORES       # 200
PER_GRAPH = NODES_PER_CORE // B    # 25

F32 = mybir.dt.float32
BF16 = mybir.dt.bfloat16
F8 = mybir.dt.float8e4

DEBUG_STAGE = os.environ.get("KERNEL_DEBUG_STAGE") or None

# node k-tiles over the 1600-node dim
KT = [(t * 128, min(128, N - t * 128)) for t in range((N + 127) // 128)]  # 13
NKT = len(KT)

GAW = 514         # gather payload width: 512 feats | ones | asrc2
GAWP = 528        # g1a SBUF row pitch: 16B-aligned for DoubleRow weight APs
PTP = 208         # pt2 SBUF row pitch: 16B-aligned for DoubleRow moving APs

_BF = ml_dtypes.bfloat16


def _bf(x):
    return np.ascontiguousarray(x.astype(_BF))


def _f32(x):
    return np.ascontiguousarray(x.astype(np.float32))


def _ap_cols(ap, start, stride, count):
    """Sub-AP selecting `count` columns with `stride` from a 2D [P, F] AP."""
    return bass.AP(
        tensor=ap.tensor,
        offset=ap.offset + start * ap.ap[-1][0],
        ap=[ap.ap[0], [ap.ap[-1][0] * stride, count]],
    )


def _dram_bcast(handle, n_part, offset, stride, count):
    """DRAM read AP replicating a strided 1-D slice across n_part partitions."""
    return bass.AP(
        tensor=handle, offset=offset, ap=[[0, n_part], [stride, count]]
    )


# ---------------------------------------------------------------- program
def build_program():
    nc = bacc.Bacc("TRN2", num_devices=NCORES)

    # ---- I/O ----
    d_xT = nc.dram_tensor("xT", [100, 2, N], BF16, kind="ExternalInput")
    d_xTd = nc.dram_tensor("xTd", [100, 2, NODES_PER_CORE], BF16, kind="ExternalInput")
    d_W1 = nc.dram_tensor("W1", [100, 2, D1], BF16, kind="ExternalInput")
    d_Wa = nc.dram_tensor("Wa", [100, 2, 16], BF16, kind="ExternalInput")
    d_Ct1 = nc.dram_tensor("Ct1", [128, NKT, NODES_PER_CORE], BF16, kind="ExternalInput")
    d_Ct2 = nc.dram_tensor("Ct2", [128, NKT, NODES_PER_CORE], BF16, kind="ExternalInput")
    d_b1 = nc.dram_tensor("b1", [D1], F32, kind="ExternalInput")
    d_Wb = nc.dram_tensor("Wb", [2, D1], F32, kind="ExternalInput")
    d_WfcT = nc.dram_tensor("WfcT", [128, 100, ROI], BF16, kind="ExternalInput")
    d_sel = nc.dram_tensor("sel", [128, B], F32, kind="ExternalInput")
    d_Ebc = nc.dram_tensor("Ebc", [16, HIN, 128], BF16, kind="ExternalInput")
    d_E2 = nc.dram_tensor("E2", [2, 2, 128], BF16, kind="ExternalInput")
    d_eye100 = nc.dram_tensor("eye100", [100, 100], F32, kind="ExternalInput")
    d_bnsc8 = nc.dram_tensor("bnsc8", [B, ROI], F32, kind="ExternalInput")
    d_bnsh8 = nc.dram_tensor("bnsh8", [B, ROI], F32, kind="ExternalInput")
    d_fc2w8 = nc.dram_tensor("fc2w8", [B, 2, ROI], F32, kind="ExternalInput")
    d_fc2b8 = nc.dram_tensor("fc2b8", [B, 2], F32, kind="ExternalInput")
    d_out = nc.dram_tensor("logits", [B, 2], F32, kind="ExternalOutput")
    d_dbg = nc.dram_tensor("dbg", [128, 800], F32, kind="ExternalOutput")

    # ---- collective buffers ----
    d_ag_in = nc.dram_tensor("ag_in", [NODES_PER_CORE, GAW], F8, kind="Internal")
    d_ag_out = nc.dram_tensor(
        "ag_out", [N, GAW], F8, kind="Internal", addr_space="Shared"
    )
    d_warm_in = nc.dram_tensor("warm_in", [1, 8], F8, kind="Internal")
    d_warm_out = nc.dram_tensor(
        "warm_out", [NCORES, 8], F8, kind="Internal", addr_space="Shared"
    )
    d_ar_in = nc.dram_tensor("ar_in", [B, ROI], F32, kind="Internal")
    d_ar_out = nc.dram_tensor(
        "ar_out", [B, ROI], F32, kind="Internal", addr_space="Shared"
    )

    groups = [list(range(NCORES))]

    with TileContext(nc) as tc:
        _build_body(nc, tc, locals())

    nc.finalize()

    in_names = [
        "xT", "xTd", "W1", "Wa", "Ct1", "Ct2", "b1", "Wb",
        "WfcT", "sel", "Ebc", "E2", "eye100", "bnsc8", "bnsh8", "fc2w8", "fc2b8",
    ]
    return nc, in_names


def _build_body(nc, tc, d):
    from contextlib import ExitStack

    d_xT = d["d_xT"]; d_xTd = d["d_xTd"]; d_W1 = d["d_W1"]; d_Wa = d["d_Wa"]
    d_Ct1 = d["d_Ct1"]; d_Ct2 = d["d_Ct2"]; d_b1 = d["d_b1"]
    d_Wb = d["d_Wb"]; d_WfcT = d["d_WfcT"]; d_sel = d["d_sel"]
    d_bnsc8 = d["d_bnsc8"]; d_bnsh8 = d["d_bnsh8"]
    d_fc2w8 = d["d_fc2w8"]; d_fc2b8 = d["d_fc2b8"]; d_out = d["d_out"]
    d_ag_in = d["d_ag_in"]; d_ag_out = d["d_ag_out"]
    d_warm_in = d["d_warm_in"]; d_warm_out = d["d_warm_out"]
    d_Ebc = d["d_Ebc"]; d_E2 = d["d_E2"]; d_eye100 = d["d_eye100"]
    d_ar_in = d["d_ar_in"]; d_ar_out = d["d_ar_out"]
    d_dbg = d["d_dbg"]
    groups = d["groups"]

    ACT = mybir.ActivationFunctionType
    ALU = mybir.AluOpType
    DR = mybir.MatmulPerfMode.DoubleRow

    def _dbg_out(work, src_ap):
        dbg = work.tile([B, 2], F32, tag="dbgo", name="dbgo")
        nc.vector.tensor_copy(dbg, src_ap)
        nc.sync.dma_start(out=d_out[:], in_=dbg)

    with ExitStack() as ctx:
        singles = ctx.enter_context(tc.tile_pool(name="singles", bufs=1))
        work = ctx.enter_context(tc.tile_pool(name="work", bufs=3))

        # ------------------------------------------------ static loads
        ones_row = singles.tile([1, 128], BF16)
        nc.vector.memset(ones_row, 1.0)
        # No warm-up collective: the CC subsystem takes ~78us from kernel
        # start to become ready regardless of trigger time, and each mesh
        # blocks the next trigger for ~20us after it ends.  Layer 1 finishes
        # right around the ready point, so the real AllGather IS the warm-up.

        # phase-A-critical loads split across the two HWDGE rings; each ring
        # drains in issue order, so the critical tensors arrive first.
        # Ct2 / WfcT (needed ~100us later) are issued LAST so they don't
        # crowd HBM bandwidth while phase A waits on xT/W1/Ct1.
        xTd = singles.tile([100, 2, NODES_PER_CORE], BF16)
        nc.sync.dma_start(out=xTd[:], in_=d_xTd[:])
        Wa = singles.tile([100, 2, 16], BF16)
        nc.sync.dma_start(out=Wa[:], in_=d_Wa[:])
        xT = singles.tile([100, 2, N], BF16)           # x^T k-tiles (K=200=2x100)
        for ci, (c0, c1) in enumerate(((0, 512), (512, 1024), (1024, 1536), (1536, N))):
            eng = nc.sync if ci % 2 == 0 else nc.scalar
            eng.dma_start(out=xT[:, :, c0:c1], in_=d_xT[:, :, c0:c1])
        W1 = singles.tile([100, 2, D1], BF16)
        nc.sync.dma_start(out=W1[:], in_=d_W1[:])

        Ct1 = singles.tile([128, NKT, NODES_PER_CORE], BF16)
        Ct2 = singles.tile([128, NKT, NODES_PER_CORE], BF16)
        nc.sync.dma_start(out=Ct1[:, 0:7, :], in_=d_Ct1[:, 0:7, :])
        nc.scalar.dma_start(out=Ct1[:, 7:NKT, :], in_=d_Ct1[:, 7:NKT, :])

        b1b = singles.tile([128, D1], F32)  # b1 broadcast across partitions
        nc.sync.dma_start(out=b1b, in_=_dram_bcast(d_b1, 128, 0, 1, D1))
        Wbb = singles.tile([128, 2, D1], F32)  # wsrc2 / wdst2 broadcast
        nc.sync.dma_start(
            out=Wbb,
            in_=bass.AP(tensor=d_Wb, offset=0, ap=[[0, 128], [D1, 2], [1, D1]]),
        )
        WfcT = singles.tile([128, 100, ROI], BF16)
        sel = singles.tile([128, B], F32)
        Ebc = singles.tile([16, HIN, 128], BF16)
        nc.sync.dma_start(out=Ebc[:], in_=d_Ebc[:])
        E2 = singles.tile([2, 2, 128], BF16)
        nc.sync.dma_start(out=E2[:], in_=d_E2[:])
        eye100 = singles.tile([100, 100], F32)
        nc.scalar.dma_start(out=eye100[:], in_=d_eye100[:])
        bnsc8 = singles.tile([B, ROI], F32)
        bnsh8 = singles.tile([B, ROI], F32)
        fc2w8 = singles.tile([B, 2, ROI], F32)
        fc2b8 = singles.tile([B, 2], F32)

        # small tail constants early (cheap), then the big late tensors last.
        # The late bulk (Ct2, WfcT -- 5.8 MB needed only ~100us in) rides the
        # scalar ring ONLY: the sync ring must stay clear for mid-kernel DMAs
        # (wdst round trip, gather payload, g1a loads) which would otherwise
        # queue behind these in ring-FIFO order.
        nc.scalar.dma_start(out=sel[:], in_=d_sel[:])
        nc.scalar.dma_start(out=bnsc8[:], in_=d_bnsc8[:])
        nc.scalar.dma_start(out=bnsh8[:], in_=d_bnsh8[:])
        nc.scalar.dma_start(out=fc2w8[:], in_=d_fc2w8[:])
        nc.scalar.dma_start(out=fc2b8[:], in_=d_fc2b8[:])
        nc.scalar.dma_start(out=Ct2[:], in_=d_Ct2[:])
        for wc in range(4):
            nc_slice = slice(wc * 25, (wc + 1) * 25)
            nc.scalar.dma_start(out=WfcT[:, nc_slice, :], in_=d_WfcT[:, nc_slice, :])

        # ------------------------------------------------ phase A: asdT1 + h1
        h1s = singles.tile([128, NKT, HIN, HID + 1], BF16)
        nc.vector.memset(h1s[:, :, :, HID : HID + 1], 1.0)
        # PE warm-up burst: junk matmuls during the input-DMA wait push
        # the HAM clock gate up before the real matmuls start.
        with tc.tile_pool(name="pW", bufs=1, space="PSUM") as pW:
            psW = pW.tile([128, 128], F32, tag="psW")
            for i in range(24):
                nc.tensor.matmul(
                    psW, ones_row, ones_row, start=(i == 0), stop=(i == 23)
                )
        asd1 = singles.tile([128, NKT, HIN], F32)   # asrc1 for all nodes
        nc.vector.memset(asd1[64:128, NKT - 1, :], 0.0)  # pad tile garbage guard
        ue = singles.tile([128, NKT, HIN], F32)     # exp(asrc1)
        ue2 = singles.tile([128, NKT, HIN], F32)    # exp(0.2*asrc1)
        wb = singles.tile([128, HIN, NODES_PER_CORE], BF16)  # exp(-0.8*adst1) bcast
        with tc.tile_pool(name="pA", bufs=2, space="PSUM") as pA:
            # asdT1 = Wa^T @ x[D_j]^T  (row h = asrc1_h, row 8+h = adst1_h)
            ps_asdT = pA.tile([16, NODES_PER_CORE], F32)
            for kc in range(2):
                nc.tensor.matmul(
                    ps_asdT, Wa[:, kc, :], xTd[:, kc, :],
                    start=(kc == 0), stop=(kc == 1),
                )
            # w_h[d] = exp(-0.8 * adst1_h[d]); broadcast across partitions by
            # 8 selector matmuls on the PE (no DRAM round trip)
            wloc = work.tile([16, NODES_PER_CORE], BF16, tag="wloc")
            nc.scalar.activation(wloc, ps_asdT, ACT.Exp, scale=-0.8)
            for h in range(HIN):
                ps_wb = pA.tile([128, NODES_PER_CORE], F32, tag="ps_wb")
                nc.tensor.matmul(ps_wb, Ebc[:, h, :], wloc)
                nc.vector.tensor_copy(wb[:, h, :], ps_wb)

            # attention-score matmuls first so ue/ue2 (which gate the whole
            # layer-1 DVE producer chain) are ready before the h1 matmuls run
            for k, (k0, pk) in enumerate(KT):
                ps_a = pA.tile([128, HIN], F32, tag="ps_a")
                for kc in range(2):
                    nc.tensor.matmul(
                        ps_a[0:pk, :],
                        xT[:, kc, k0 : k0 + pk],
                        Wa[:, kc, 0:HIN],
                        start=(kc == 0),
                        stop=(kc == 1),
                    )
                nc.scalar.copy(asd1[0:pk, k, :], ps_a[0:pk, :])
                if k in (3, 7, 11, 12):
                    ka = {3: 0, 7: 4, 11: 8, 12: 12}[k]
                    nc.scalar.activation(
                        ue[:, ka : k + 1, :], asd1[:, ka : k + 1, :], ACT.Exp
                    )
                    nc.scalar.activation(
                        ue2[:, ka : k + 1, :], asd1[:, ka : k + 1, :],
                        ACT.Exp, scale=NEG,
                    )

            # h1 = x @ W1;  h1s[:, k, h, 64] stays 1.0 (denominator col)
            for k, (k0, pk) in enumerate(KT):
                ps_h = pA.tile([128, D1], F32, tag="ps_h")
                for kc in range(2):
                    nc.tensor.matmul(
                        ps_h[0:pk, :],
                        xT[:, kc, k0 : k0 + pk],
                        W1[:, kc, :],
                        start=(kc == 0),
                        stop=(kc == 1),
                    )
                pa = ps_h[0:pk, :]
                nc.scalar.copy(
                    h1s[0:pk, k, :, 0:HID],
                    bass.AP(
                        tensor=pa.tensor,
                        offset=pa.offset,
                        ap=[pa.ap[0], [HID, HIN], [1, HID]],
                    ),
                )

        if DEBUG_STAGE == "h1":
            dbh = work.tile([128, 536], F32, tag="dbh", name="dbh")
            nc.vector.tensor_copy(dbh[:, 0:520], h1s[:, 0, :, :])
            nc.vector.tensor_copy(dbh[:, 520:528], ue[:, 0, :])
            nc.vector.tensor_copy(dbh[:, 528:536], ue2[:, 0, :])
            nc.sync.dma_start(out=d_dbg[:, 0:536], in_=dbh)
            _dbg_out(work, ue[0:B, 0, 0:2])
            return

        # ------------------------------------------------ layer-1 attention
        g1f = singles.tile([100, 2, D1], F32)  # post-attention hidden state
        asd2acc = singles.tile([100, 2, 2, 3], F32)  # [m, vi, chunk] partial dots
        g1o1 = singles.tile([100, 2, 256], F8)
        g1o2 = singles.tile([100, 2, 258], F8)
        nc.vector.memset(g1o2[:, :, 256:257], 1.0)   # ones col for L2 denominator
        with tc.tile_pool(name="pL1", bufs=2, space="PSUM") as pL1:
            for h in range(HIN):
                # pair-space attention weights:
                #   pt[s,d] = Ct1[s,d] * max(u[s], u2[s]*w[d])
                m2 = work.tile([128, NKT, NODES_PER_CORE], BF16, tag="m2", bufs=2)
                for k, (k0, pk) in enumerate(KT):
                    nc.vector.tensor_scalar(
                        m2[0:pk, k, :],
                        wb[0:pk, h, :],
                        ue2[0:pk, k, h : h + 1],
                        ue[0:pk, k, h : h + 1],
                        ALU.mult,
                        ALU.max,
                    )
                pt = work.tile([128, NKT, NODES_PER_CORE], BF16, tag="pt", bufs=2)
                nc.vector.tensor_mul(pt[:], m2[:], Ct1[:])

                psA = pL1.tile([100, HID + 1], F32, tag="psA")
                psB = pL1.tile([100, HID + 1], F32, tag="psB")
                for k, (k0, pk) in enumerate(KT):
                    nc.tensor.matmul(
                        psA,
                        pt[0:pk, k, 0:100],
                        h1s[0:pk, k, h, :],
                        start=(k == 0),
                        stop=(k == NKT - 1),
                    )
                    nc.tensor.matmul(
                        psB,
                        pt[0:pk, k, 100:200],
                        h1s[0:pk, k, h, :],
                        start=(k == 0),
                        stop=(k == NKT - 1),
                    )
                if DEBUG_STAGE == "l1m":
                    if h == 0:
                        _dbg_out(work, psA[0:B, 0:2])
                    continue
                for m, ps in ((0, psA), (1, psB)):
                    rec = work.tile([100, 1], F32, tag="rec")
                    nc.vector.reciprocal(rec, ps[:, HID : HID + 1])
                    # divide by softmax denominator on the scalar engine
                    nc.scalar.activation(
                        g1f[:, m, h * HID : (h + 1) * HID],
                        ps[:, 0:HID],
                        ACT.Copy,
                        scale=rec[:, 0:1],
                    )
                if h in (3, 6, HIN - 1):
                    # completed feature chunk: +b1, ELU, asd2 partial dots,
                    # then DMA into the gather payload.  Chunked at heads
                    # 3/6/7 so the tail chunk (64 cols) before the AllGather
                    # trigger is short.
                    ci = (3, 6, 7).index(h)
                    c0, c1 = ((0, 256), (256, 448), (448, 512))[ci]
                    cw = c1 - c0
                    for m in range(2):
                        vv = g1f[:, m, c0:c1]
                        nc.vector.tensor_add(vv, vv, b1b[0:100, c0:c1])
                        eneg = work.tile([100, 256], F32, tag="eneg")
                        en = eneg[:, 0:cw]
                        if ci < 2:
                            rpos = work.tile([100, 256], F32, tag="rpos")
                            rp = rpos[:, 0:cw]
                            nc.scalar.activation(en, vv, ACT.Relu, scale=-1.0)
                            nc.scalar.activation(en, en, ACT.Exp, scale=-1.0)
                            nc.scalar.activation(rp, vv, ACT.Relu)
                            nc.vector.scalar_tensor_tensor(
                                vv, en, -1.0, rp, ALU.add, ALU.add
                            )
                        else:
                            nc.vector.tensor_scalar_min(en, vv, 0.0)
                            nc.scalar.activation(en, en, ACT.Exp)
                            nc.vector.tensor_scalar(vv, vv, 0.0, -1.0, ALU.max, ALU.add)
                            nc.vector.tensor_add(vv, vv, en)
                        # partial asd2 dots over this feature chunk
                        sc2 = work.tile([100, 256], F32, tag="sc2")
                        for vi in range(2):
                            nc.vector.scalar_tensor_tensor(
                                sc2[:, 0:cw],
                                vv,
                                1.0,
                                Wbb[0:100, vi, c0:c1],
                                ALU.mult,
                                ALU.mult,
                                accum_out=asd2acc[:, m, vi, ci : ci + 1],
                            )
                        if ci == 0:
                            nc.scalar.copy(g1o1[:, m, :], vv)
                            nc.sync.dma_start(
                                out=d_ag_in[m * 100 : (m + 1) * 100, 0:256],
                                in_=g1o1[:, m, :],
                            )
                        else:
                            nc.scalar.copy(g1o2[:, m, c0 - 256 : c1 - 256], vv)
                            if ci == 1:
                                nc.sync.dma_start(
                                    out=d_ag_in[m * 100 : (m + 1) * 100, 256:448],
                                    in_=g1o2[:, m, 0:192],
                                )

        if DEBUG_STAGE == "l1m":
            return

        # ------------------------------------------------ g1 post: asd2 + AG2
        adst2c = singles.tile([100, 2], F32)
        for m in range(2):
            asd2 = work.tile([100, 2], F32, tag="asd2")
            nc.vector.tensor_add(
                asd2, asd2acc[:, m, :, 0], asd2acc[:, m, :, 1]
            )
            nc.vector.tensor_add(asd2, asd2, asd2acc[:, m, :, 2])
            # asrc2 -> gather payload col 257
            nc.vector.tensor_copy(g1o2[:, m, 257:258], asd2[:, 0:1])
            nc.vector.tensor_copy(adst2c[:, m : m + 1], asd2[:, 1:2])
        # the only sync-ring DMAs between layer-1 end and the collective
        # trigger: the two final payload chunks
        for m in range(2):
            nc.sync.dma_start(
                out=d_ag_in[m * 100 : (m + 1) * 100, 448:GAW],
                in_=g1o2[:, m, 192:258],
            )

        if DEBUG_STAGE == "l1":
            nc.sync.dma_start(out=d_dbg[0:100, 0:512], in_=g1f[:, 0, :])
            _dbg_out(work, g1f[0:B, 0, 0:2])
            return

        nc.gpsimd.collective_compute(
            "AllGather",
            ALU.bypass,
            replica_groups=groups,
            ins=[d_ag_in[:]],
            outs=[d_ag_out[:]],
        )

        # own shard's w2 = exp(-0.8*adst2) broadcast across partitions:
        # transpose the adst2 columns on the PE, exponentiate the row, then
        # two selector matmuls -- all during the collective (PE is idle).
        wb2 = singles.tile([128, NODES_PER_CORE], BF16)
        with tc.tile_pool(name="pT2", bufs=1, space="PSUM") as pT2:
            ps_t = pT2.tile([2, 100], F32, tag="ps_t")
            nc.tensor.transpose(ps_t, adst2c[:, 0:2], eye100)
            w2row = work.tile([2, 100], BF16, tag="w2row")
            nc.scalar.activation(w2row, ps_t, ACT.Exp, scale=-0.8)
            for m in range(2):
                ps_w2 = pT2.tile([128, 100], F32, tag="ps_w2")
                nc.tensor.matmul(ps_w2, E2[:, m, :], w2row)
                nc.vector.tensor_copy(wb2[:, m * 100 : (m + 1) * 100], ps_w2)

        # gathered hidden state: g1a [128, k, 514] = [feat 512 | ones | asrc2].
        # Read in 7 pair-of-tiles DMAs (fewer descriptor generations than 13)
        # and extract asrc2 -> Ue/Ue2 per pair so layer-2 starts on tile 0
        # while later pairs are still in flight.
        g1a = singles.tile([128, NKT, GAWP], F8)
        asrc2f = singles.tile([128, NKT], F32)
        Ue = singles.tile([128, NKT], F32)
        Ue2 = singles.tile([128, NKT], F32)
        nc.vector.memset(asrc2f[64:128, NKT - 1 : NKT], 0.0)  # pad guard
        for p in range(7):
            k0t = 2 * p
            nkt = 2 if p < 6 else 1
            pk = 128 if p < 6 else 64
            nc.sync.dma_start(
                out=g1a[:, k0t : k0t + nkt, 0:GAW]
                if nkt == 2
                else g1a[0:pk, k0t : k0t + 1, 0:GAW],
                in_=bass.AP(
                    tensor=d_ag_out,
                    offset=k0t * 128 * GAW,
                    ap=[[GAW, 128], [128 * GAW, nkt], [1, GAW]]
                    if nkt == 2
                    else [[GAW, pk], [1, GAW]],
                ),
            )
            nc.scalar.copy(
                asrc2f[0:pk, k0t : k0t + nkt],
                g1a[0:pk, k0t : k0t + nkt, GAW - 1 : GAW],
            )
            nc.scalar.activation(
                Ue[0:pk, k0t : k0t + nkt], asrc2f[0:pk, k0t : k0t + nkt], ACT.Exp
            )
            nc.scalar.activation(
                Ue2[0:pk, k0t : k0t + nkt], asrc2f[0:pk, k0t : k0t + nkt],
                ACT.Exp, scale=NEG,
            )

        if DEBUG_STAGE == "ag":
            dba = work.tile([128, 514], F32, tag="dba", name="dba")
            nc.vector.tensor_copy(dba, g1a[:, 0, 0:GAW])
            nc.sync.dma_start(out=d_dbg[:, 0:514], in_=dba)
            _dbg_out(work, g1a[0:B, 0, 0:2])
            return

        # ------------------------------------------------ layer-2 attention
        # attention weights pt2 in fp8 so the aggregation matmuls run
        # DoubleRow (2 k-tiles per instruction, half the cycles/row).
        tnorm = singles.tile([128, 4, NODES_PER_CORE], BF16)
        with tc.tile_pool(name="pL2", bufs=1, space="PSUM") as pL2:
            m22 = work.tile([128, NKT, NODES_PER_CORE], BF16, tag="m22", bufs=1)
            pt2 = work.tile([128, NKT, PTP], F8, tag="pt2", bufs=1)
            psT = [
                pL2.tile([128, NODES_PER_CORE], F32, tag=f"psT{c}", name=f"psT{c}")
                for c in range(4)
            ]
            psD = pL2.tile([1, NODES_PER_CORE], F32, tag="psD")
            for k, (k0, pk) in enumerate(KT):
                nc.vector.tensor_scalar(
                    m22[0:pk, k, :],
                    wb2[0:pk, :],
                    Ue2[0:pk, k : k + 1],
                    Ue[0:pk, k : k + 1],
                    ALU.mult,
                    ALU.max,
                )
                nc.vector.tensor_mul(
                    pt2[0:pk, k, 0:NODES_PER_CORE], m22[0:pk, k, :], Ct2[0:pk, k, :]
                )
                if k % 2 == 1:
                    # DoubleRow pair (k-1, k): both operands fp8
                    for c in range(4):
                        nc.tensor.matmul(
                            psT[c],
                            g1a[:, k - 1 : k + 1, c * 128 : (c + 1) * 128],
                            pt2[:, k - 1 : k + 1, 0:NODES_PER_CORE],
                            start=(k == 1),
                            stop=False,
                            perf_mode=DR,
                        )
                    nc.tensor.matmul(
                        psD,
                        g1a[:, k - 1 : k + 1, D1 : D1 + 1],
                        pt2[:, k - 1 : k + 1, 0:NODES_PER_CORE],
                        start=(k == 1),
                        stop=False,
                        perf_mode=DR,
                    )
                elif k == NKT - 1:
                    for c in range(4):
                        nc.tensor.matmul(
                            psT[c],
                            g1a[0:pk, k, c * 128 : (c + 1) * 128],
                            pt2[0:pk, k, 0:NODES_PER_CORE],
                            start=False,
                            stop=True,
                        )
                    nc.tensor.matmul(
                        psD,
                        g1a[0:pk, k, D1 : D1 + 1],
                        pt2[0:pk, k, 0:NODES_PER_CORE],
                        start=False,
                        stop=True,
                    )

            # denominator reciprocal, broadcast across partitions via PE
            d2 = work.tile([1, NODES_PER_CORE], F32, tag="d2")
            nc.vector.reciprocal(d2, psD[0:1, :])
            d2b = work.tile([1, NODES_PER_CORE], BF16, tag="d2b")
            nc.vector.tensor_copy(d2b, d2)
            ps_rb = pL2.tile([128, NODES_PER_CORE], F32, tag="ps_rb")
            nc.tensor.matmul(ps_rb, ones_row, d2b)
            rb = work.tile([128, NODES_PER_CORE], F32, tag="rb")
            nc.scalar.copy(rb, ps_rb)

            # normalized layer-2 message sums (pre-W2; W2 folded into WfcT).
            # chunk 0 first so FC1's first column group can start while the
            # other chunks normalize.
            for c in range(4):
                nc.vector.tensor_mul(tnorm[:, c, :], psT[c], rb)

                if DEBUG_STAGE == "l2" and c == 0:
                    break

        if DEBUG_STAGE == "l2":
            dbo = work.tile([128, 600], F32, tag="dbo", name="dbo")
            nc.vector.tensor_copy(dbo[:, 0:200], tnorm[:, 0, :])
            nc.sync.dma_start(out=d_dbg[:, 0:600], in_=dbo)
            _dbg_out(work, tnorm[0:B, 0, 0:2])
            return

        # ------------------------------------------------ FC1 partial + AllReduce
        # 100 k-slice matmuls with M=8 packed 4-wide into PE column groups;
        # the 4 groups are summed on-chip (selector matmul) before a small
        # [8,200] AllReduce.
        with tc.tile_pool(name="pFC", bufs=1, space="PSUM") as pFC:
            psZ4 = pFC.tile([128, ROI], F32, tag="psZ4")
            order = [c for fcch in range(4) for c in range(fcch, 100, 4)]
            counts = [0, 0, 0, 0]
            for i, c in enumerate(order):
                j = i % 4
                l = c // 4
                fcch = c % 4
                stat = _ap_cols(tnorm[:, fcch, :], l, PER_GRAPH, B)
                nc.tensor.matmul(
                    psZ4[32 * j : 32 * j + B, :],
                    stat,
                    WfcT[:, c, :],
                    start=(counts[j] == 0),
                    stop=(counts[j] == 24),
                    tile_position=(0, 32 * j),
                    skip_group_check=True,
                )
                counts[j] += 1
            # sum the 4 column groups on-chip: psZs = sel^T @ zsb
            zsb = work.tile([128, ROI], F32, tag="zsb")
            nc.vector.tensor_copy(zsb, psZ4)
            psZs = pFC.tile([B, ROI], F32, tag="psZs")
            nc.tensor.matmul(psZs, sel, zsb)
            zs8 = work.tile([B, ROI], F32, tag="zs8")
            nc.vector.tensor_copy(zs8, psZs)
            nc.sync.dma_start(out=d_ar_in[:], in_=zs8)

            if DEBUG_STAGE == "fc":
                nc.sync.dma_start(out=d_dbg[0:B, 0:200], in_=zs8[0:B, :])
                _dbg_out(work, zs8[0:B, 0:2])
                return

            nc.gpsimd.collective_compute(
                "AllReduce",
                ALU.add,
                replica_groups=groups,
                ins=[d_ar_in[:]],
                outs=[d_ar_out[:]],
            )

            # ------------------------------------------------ BN + ELU + FC2
            # contiguous [8,200] read; BN via host-pre-broadcast scale/shift;
            # FC2 as two DVE accumulation ops (no transpose round-trip).
            zt = work.tile([B, ROI], F32, tag="zt")
            nc.sync.dma_start(out=zt, in_=d_ar_out[:])
            nc.vector.tensor_mul(zt, zt, bnsc8)
            nc.vector.tensor_add(zt, zt, bnsh8)
            en = work.tile([B, ROI], F32, tag="en")
            nc.vector.tensor_scalar_min(en, zt, 0.0)
            nc.scalar.activation(en, en, ACT.Exp)
            nc.vector.tensor_scalar(zt, zt, 0.0, -1.0, ALU.max, ALU.add)
            nc.vector.tensor_add(zt, zt, en)
            lg = work.tile([B, 2], F32, tag="lg")
            junk = work.tile([B, ROI], F32, tag="junk")
            for c in range(2):
                nc.vector.scalar_tensor_tensor(
                    junk, zt, 1.0, fc2w8[:, c, :], ALU.mult, ALU.mult,
                    accum_out=lg[:, c : c + 1],
                )
            lsb = work.tile([B, 2], F32, tag="lsb")
            nc.vector.tensor_add(lsb, lg, fc2b8)
            nc.sync.dma_start(out=d_out[:], in_=lsb)


# ---------------------------------------------------------------- host side
def _prepare_inputs(x, edge_index, W1, a1_src, a1_dst, b1, W2, a2_src, a2_dst,
                    b2, fc1_w, fc1_b, bn_g, bn_b, bn_m, bn_v, fc2_w, fc2_b):
    x = np.asarray(x, np.float32)
    ei = np.asarray(edge_index)
    src, dst = ei[0].astype(np.int64), ei[1].astype(np.int64)
    C = np.bincount(dst * N + src, minlength=N * N).reshape(N, N).astype(np.float32)
    assert C.max() < 256, "edge multiplicity too large for bf16"

    # pi-order: core j owns, for each graph g, within-graph nodes [25j, 25j+25)
    D = [
        np.array(
            [g * ROI + PER_GRAPH * j + k for g in range(B) for k in range(PER_GRAPH)],
            np.int64,
        )
        for j in range(NCORES)
    ]
    perm = np.concatenate(D)

    W1 = np.asarray(W1, np.float32)
    a1_src = np.asarray(a1_src, np.float32)
    a1_dst = np.asarray(a1_dst, np.float32)
    W2 = np.asarray(W2, np.float32)
    a2_src = np.asarray(a2_src, np.float32)
    a2_dst = np.asarray(a2_dst, np.float32)
    fc1_w = np.asarray(fc1_w, np.float32)
    b2 = np.asarray(b2, np.float32)

    # Wa[:, h] = W1[:, 64h:64h+64] @ a1_src[h] ; cols 8..16 same with a1_dst
    W1r = W1.reshape(ROI, HIN, HID)
    Wa = np.concatenate(
        [
            np.einsum("rhf,hf->rh", W1r, a1_src),
            np.einsum("rhf,hf->rh", W1r, a1_dst),
        ],
        axis=1,
    )  # [200, 16]
    Wb = np.stack([W2 @ a2_src[0], W2 @ a2_dst[0]], axis=0)  # [2, 512]

    # z = tnorm_flat @ WF^T + zb;  WF[o,i,f0] = sum_f fc1_wr[o,i,f] W2[f0,f]
    fc1_wr = fc1_w.reshape(ROI, ROI, D1)  # [o, node-in-graph, feat]
    zb = fc1_wr.reshape(ROI, -1) @ np.tile(b2, ROI)  # [200] b2 contribution

    bnscale = np.asarray(bn_g, np.float32) / np.sqrt(np.asarray(bn_v, np.float32) + 1e-5)
    bnshift = (
        np.asarray(bn_b, np.float32)
        + (np.asarray(fc1_b, np.float32) + zb - np.asarray(bn_m, np.float32)) * bnscale
    )

    def _ksw(arr2d, P):
        """[K*P?, F] -> [P, K, F] partition-contiguous swizzle (rows r = k*P + p),
        zero-padding rows beyond the array."""
        R, F = arr2d.shape
        K = (R + P - 1) // P
        out = np.zeros((P, K, F), arr2d.dtype)
        for k in range(K):
            r0, r1 = k * P, min((k + 1) * P, R)
            out[0 : r1 - r0, k, :] = arr2d[r0:r1]
        return out

    xT = _bf(_ksw(x.T, 100))                       # [100, 2, 1600]
    W1_b = _bf(_ksw(W1, 100))                      # [100, 2, 512]
    Wa_b = _bf(_ksw(Wa, 100))                      # [100, 2, 16]

    Ebc_h = np.zeros((16, HIN, 128), np.float32)
    for h in range(HIN):
        Ebc_h[8 + h, h, :] = 1.0
    Ebc_h = _bf(Ebc_h)
    E2_h = np.zeros((2, 2, 128), np.float32)
    for m in range(2):
        E2_h[m, m, :] = 1.0
    E2_h = _bf(E2_h)
    eye100 = np.eye(100, dtype=np.float32)
    sel = np.zeros((128, B), np.float32)
    for j in range(4):
        for g in range(B):
            sel[32 * j + g, g] = 1.0
    bnsc8 = _f32(np.tile(bnscale[None, :], (B, 1)))       # [8, 200]
    bnsh8 = _f32(np.tile(bnshift[None, :], (B, 1)))       # [8, 200]
    fc2w8 = _f32(np.tile(np.asarray(fc2_w, np.float32)[None, :, :], (B, 1, 1)))
    fc2b8 = _f32(np.tile(np.asarray(fc2_b, np.float32)[None, :], (B, 1)))

    in_maps = []
    for j in range(NCORES):
        Dj = D[j]
        Ct1 = _bf(_ksw(C[Dj, :].T, 128))           # [128, 13, 200]
        Ct2 = _bf(_ksw(C[np.ix_(Dj, perm)].T, 128))
        xTd = _bf(_ksw(x[Dj, :].T, 100))           # [100, 2, 200]
        # FC1 weight slice with W2 folded in:
        # WF[o, i, f0] = fc1_wr[o, 25j+i, :] @ W2[f0, :]^T
        fc1c = fc1_wr[:, PER_GRAPH * j : PER_GRAPH * (j + 1), :]  # [200, 25, 512]
        WF = (fc1c.reshape(-1, D1) @ W2.T).reshape(ROI, PER_GRAPH, D1)
        WfcT = _bf(
            _ksw(WF.transpose(1, 2, 0).reshape(PER_GRAPH * D1, ROI), 128)
        )                                          # [128, 100, 200]
        in_maps.append(
            {
                "xT": xT,
                "xTd": xTd,
                "W1": W1_b,
                "Wa": Wa_b,
                "Ct1": Ct1,
                "Ct2": Ct2,
                "b1": _f32(np.asarray(b1)),
                "Wb": _f32(Wb),
                "WfcT": WfcT,
                "sel": sel,
                "Ebc": Ebc_h,
                "E2": E2_h,
                "eye100": eye100,
                "bnsc8": bnsc8,
                "bnsh8": bnsh8,
                "fc2w8": fc2w8,
                "fc2b8": fc2b8,
            }
        )
    return in_maps


_CACHE = {}


def kernel(**inputs):
    if "nc" not in _CACHE:
        nc, in_names = build_program()
        _CACHE["nc"] = nc
        _CACHE["in_names"] = in_names
    nc = _CACHE["nc"]
    in_maps = _prepare_inputs(**inputs)
    res = run_bass_kernel_spmd(nc, in_maps, core_ids=list(range(NCORES)))
    _CACHE["last_results"] = res
    return np.asarray(res.results[0]["logits"], np.float32)


if __name__ == "__main__":
    import reference

    inp = {k: np.asarray(v) for k, v in reference.setup_inputs().items()}
    out = kernel(**inp)
    exp = np.asarray(reference.reference(**inp))
    err = np.abs(out - exp).max() / (np.abs(exp).max() + 1e-30)
    print("out:", out)
    print("exp:", exp)
    print("rel err:", err)
